# revision 1
# baseline (speedup 1.0000x reference)
"""Trainium2 Bass kernel for nn_EquivariantModel (e3nn-style equivariant net).

Architecture (per batch row): two blocks of
  {o3.Linear x2 -> FullyConnectedTensorProduct('Mx0e+Mx1o' ^2 -> 128x0e+128x1o)
   -> learnable tanh gate -> o3.Linear}, then a final o3.Linear.

Strategy: data-parallel over batch (8 cores x 1024 rows), feature-major
activations [feature, batch] on-device.  The tensor product is computed as
z[(u,v), b] = f1[u,b] * f2[v,b] (f16, formed on DVE with PE-assisted
partition broadcast of the f1 factor), followed by PSUM-accumulated
matmuls z^T @ W[(u,v), w] over k-tiles.  Linears/gates run in fp32.
All normalization constants are folded into the weights host-side.
"""

import sys
import numpy as np

if '/opt/trn_rl_repo' not in sys.path:
    sys.path.insert(0, '/opt/trn_rl_repo')

B, M_IN, M_HID = 8192, 64, 128
N_CORES = 8
BC = B // N_CORES            # batch per core
CH = 512                     # chunk of batch processed per matmul group
NCH = BC // CH
TANH_GAIN = 1.5927116870880127

F32 = None  # set after mybir import
BF16 = None

_CACHE = {}


def _build_program(repeat=1):
    import concourse.mybir as mybir
    import concourse.tile as tile
    from concourse import bacc
    from contextlib import ExitStack

    f32 = mybir.dt.float32
    f16 = mybir.dt.float16

    nc = bacc.Bacc("TRN2", target_bir_lowering=False)

    # ---- DRAM I/O ----
    s0 = nc.dram_tensor("s0", [64, BC], f32, kind="ExternalInput")
    v0 = nc.dram_tensor("v0", [192, BC], f32, kind="ExternalInput")  # rows i*64+u

    dram = {}
    for blk, M in (("b1", 64), ("b2", 128)):
        for nm in ("l1_w0", "l1_w1", "l2_w0", "l2_w1"):
            dram[f"{blk}_{nm}"] = nc.dram_tensor(f"{blk}_{nm}", [M, M], f32,
                                                 kind="ExternalInput")
        K = M * M
        for nm in ("ss", "vv", "sv", "vs"):
            # p-major layout: [128, K//128, 128]
            dram[f"{blk}_w_{nm}"] = nc.dram_tensor(
                f"{blk}_w_{nm}", [128, K // 128, 128], f16, kind="ExternalInput")
        for nm in ("g_ws", "g_wg", "g_wv", "o_w0", "o_w1"):
            dram[f"{blk}_{nm}"] = nc.dram_tensor(f"{blk}_{nm}", [128, 128], f32,
                                                 kind="ExternalInput")
    dram["f_w0"] = nc.dram_tensor("f_w0", [128, 64], f32, kind="ExternalInput")
    dram["f_w1"] = nc.dram_tensor("f_w1", [128, 64], f32, kind="ExternalInput")
    idm = nc.dram_tensor("idm", [128, 128], f16, kind="ExternalInput")
    di64 = nc.dram_tensor("di64", [64, 4096], f16, kind="ExternalInput")

    out_d = nc.dram_tensor("out", [256, BC], f32, kind="ExternalOutput")

    with ExitStack() as ctx:
        tc = ctx.enter_context(tile.TileContext(nc))
        consts = ctx.enter_context(tc.tile_pool(name="consts", bufs=1))
        acts = ctx.enter_context(tc.tile_pool(name="acts", bufs=1))
        wstream = ctx.enter_context(tc.tile_pool(name="wstream", bufs=2))
        bc_ps = ctx.enter_context(tc.tile_pool(name="bc_ps", bufs=4, space="PSUM"))
        acc_ps = ctx.enter_context(tc.tile_pool(name="acc_ps", bufs=1, space="PSUM"))
        bc_sb = ctx.enter_context(tc.tile_pool(name="bc_sb", bufs=2))
        z_pool = ctx.enter_context(tc.tile_pool(name="z", bufs=6))
        tmp = ctx.enter_context(tc.tile_pool(name="tmp", bufs=3))

        # ---- load constants ----
        W = {}
        for name, t in dram.items():
            if name.startswith("b1_w_"):
                w = consts.tile([128, 32, 128], f16, tag=name, name=name)
                nc.sync.dma_start(w[:], t[:])
                W[name] = w
            elif name.startswith("b2_w_"):
                W[name] = t  # streamed
            else:
                shp = list(t.shape)
                w = consts.tile(shp, f32, tag=name, name=name)
                nc.sync.dma_start(w[:], t[:])
                W[name] = w
        id_sb = consts.tile([128, 128], f16, tag="idm", name="idm")
        nc.sync.dma_start(id_sb[:], idm[:])
        di_sb = consts.tile([64, 4096], f16, tag="di64", name="di64")
        nc.sync.dma_start(di_sb[:], di64[:])

        # ---- input activations ----
        sT = acts.tile([64, BC], f32, tag="in_s", name="in_s")
        nc.sync.dma_start(sT[:], s0[:])
        vT = []
        for i in range(3):
            t = acts.tile([64, BC], f32, tag=f"in_v{i}", name=f"in_v{i}")
            nc.sync.dma_start(t[:], v0[i * 64:(i + 1) * 64, :])
            vT.append(t)

        def linear(w_sb, x_sb, Min, Mout, out_sb, out_rows=None, second_rows=None):
            """out = w^T x, feature-major; optional duplicate write to rows."""
            for c in range(NCH):
                sl = slice(c * CH, (c + 1) * CH)
                ps = bc_ps.tile([128, CH], f32, tag="bc", name="bc")
                nc.tensor.matmul(ps[:Mout], w_sb[:Min, :Mout], x_sb[:Min, sl],
                                 start=True, stop=True)
                r0 = out_rows or slice(0, Mout)
                nc.scalar.copy(out_sb[r0, sl], ps[:Mout])
                if second_rows is not None:
                    nc.scalar.copy(out_sb[second_rows, sl], ps[:Mout])

        def block(blk, U, s_in, v_in, s_out, v_out):
            """One equivariant block. s_in [U, BC] f32, v_in [3][U, BC] f32.
            Writes s_out [128, BC] f32, v_out [3][128, BC] f32."""
            V = U
            K = U * V
            KT = K // 128
            g = 128 // V

            # --- l1 / l2 linears -> bf16 operands ---
            # bcast-side factors (f1): s1b [U, BC], v1b[i] [U, BC]
            s1b = acts.tile([U, BC], f16, tag="s1b", name="s1b")
            v1b = [acts.tile([U, BC], f16, tag=f"v1b{i}", name=f"v1b{i}") for i in range(3)]
            # tile-side factors (f2), partition-replicated to 128 rows
            s2r = acts.tile([128, BC], f16, tag="s2r", name="s2r")
            v2r = [acts.tile([128, BC], f16, tag=f"v2r{i}", name=f"v2r{i}") for i in range(3)]

            dup = slice(64, 128) if g == 2 else None
            linear(W[f"{blk}_l1_w0"], s_in, U, U, s1b)
            for i in range(3):
                linear(W[f"{blk}_l1_w1"], v_in[i], U, U, v1b[i])
            linear(W[f"{blk}_l2_w0"], s_in, U, U, s2r, second_rows=dup)
            for i in range(3):
                linear(W[f"{blk}_l2_w1"], v_in[i], U, U, v2r[i], second_rows=dup)

            # --- tensor product ---
            tp_s = acts.tile([128, BC], f32, tag="tp_s", name="tp_s")
            tp_v = [acts.tile([128, BC], f32, tag=f"tp_v{i}", name=f"tp_v{i}") for i in range(3)]

            for c in range(NCH):
                sl = slice(c * CH, (c + 1) * CH)
                acc_s = acc_ps.tile([128, CH], f32, tag="acc_s", name="acc_s")
                acc_v = [acc_ps.tile([128, CH], f32, tag=f"acc_v{i}", name=f"acc_v{i}")
                         for i in range(3)]
                for kt in range(KT):
                    u0 = kt * g
                    if blk == "b1":
                        wss = W["b1_w_ss"][:, kt]
                        wvv = W["b1_w_vv"][:, kt]
                        wsv = W["b1_w_sv"][:, kt]
                        wvs = W["b1_w_vs"][:, kt]
                    else:
                        wss = wstream.tile([128, 128], f16, tag="wss", name="wss")
                        nc.sync.dma_start(wss[:], W["b2_w_ss"][:, kt])
                        wvv = wstream.tile([128, 128], f16, tag="wvv", name="wvv")
                        nc.sync.dma_start(wvv[:], W["b2_w_vv"][:, kt])
                        wsv = wstream.tile([128, 128], f16, tag="wsv", name="wsv")
                        nc.sync.dma_start(wsv[:], W["b2_w_sv"][:, kt])
                        wvs = wstream.tile([128, 128], f16, tag="wvs", name="wvs")
                        nc.sync.dma_start(wvs[:], W["b2_w_vs"][:, kt])

                    # partition-broadcast of f1 rows via selector matmul
                    if g == 2:
                        sel = di_sb[:64, 64 * u0: 64 * u0 + 128]
                    else:
                        sel = id_sb[:, u0:u0 + 1].to_broadcast((128, 128))
                    bps = bc_ps.tile([128, CH], f32, tag="bc", name="bc")
                    nc.tensor.matmul(bps, sel, s1b[:U, sl],
                                     start=True, stop=True)
                    bs = bc_sb.tile([128, CH], f16, tag="bcs", name="bcs")
                    nc.scalar.copy(bs, bps)
                    bv = []
                    for i in range(3):
                        p = bc_ps.tile([128, CH], f32, tag="bc", name="bc")
                        nc.tensor.matmul(p, sel, v1b[i][:U, sl],
                                         start=True, stop=True)
                        t = bc_sb.tile([128, CH], f16, tag=f"bcv{i}", name=f"bcv{i}")
                        nc.scalar.copy(t, p)
                        bv.append(t)

                    first = kt == 0
                    last = kt == KT - 1
                    # scalar output: ss + vv paths accumulate into acc_s
                    z = z_pool.tile([128, CH], f16, tag="z", name="z")
                    nc.vector.tensor_mul(z, bs, s2r[:, sl])
                    nc.tensor.matmul(acc_s, wss, z, start=first, stop=False)
                    for i in range(3):
                        z = z_pool.tile([128, CH], f16, tag="z", name="z")
                        nc.vector.tensor_mul(z, bv[i], v2r[i][:, sl])
                        nc.tensor.matmul(acc_s, wvv, z, start=False,
                                         stop=(last and i == 2))
                    # vector outputs: sv + vs paths
                    for i in range(3):
                        z = z_pool.tile([128, CH], f16, tag="z", name="z")
                        nc.vector.tensor_mul(z, bs, v2r[i][:, sl])
                        nc.tensor.matmul(acc_v[i], wsv, z, start=first, stop=False)
                    for i in range(3):
                        z = z_pool.tile([128, CH], f16, tag="z", name="z")
                        nc.vector.tensor_mul(z, bv[i], s2r[:, sl])
                        nc.tensor.matmul(acc_v[i], wvs, z, start=False, stop=last)

                nc.vector.tensor_copy(tp_s[:, sl], acc_s)
                for i in range(3):
                    nc.vector.tensor_copy(tp_v[i][:, sl], acc_v[i])

            # --- gate ---
            tanh_s = acts.tile([128, BC], f32, tag="tanh_s", name="tanh_s")
            gated_v = [acts.tile([128, BC], f32, tag=f"gated_v{i}", name=f"gated_v{i}")
                       for i in range(3)]
            for c in range(NCH):
                sl = slice(c * CH, (c + 1) * CH)
                ps = bc_ps.tile([128, CH], f32, tag="bc", name="bc")
                nc.tensor.matmul(ps, W[f"{blk}_g_ws"], tp_s[:, sl],
                                 start=True, stop=True)
                nc.scalar.activation(tanh_s[:, sl], ps,
                                     mybir.ActivationFunctionType.Tanh)
                psg = bc_ps.tile([128, CH], f32, tag="bc", name="bc")
                nc.tensor.matmul(psg, W[f"{blk}_g_wg"], tp_s[:, sl],
                                 start=True, stop=True)
                tg = tmp.tile([128, CH], f32, tag="tanh_g", name="tanh_g")
                nc.scalar.activation(tg, psg,
                                     mybir.ActivationFunctionType.Tanh)
                for i in range(3):
                    psv = bc_ps.tile([128, CH], f32, tag="bc", name="bc")
                    nc.tensor.matmul(psv, W[f"{blk}_g_wv"], tp_v[i][:, sl],
                                     start=True, stop=True)
                    nc.vector.tensor_mul(gated_v[i][:, sl], psv, tg)

            # --- out linear ---
            linear(W[f"{blk}_o_w0"], tanh_s, 128, 128, s_out)
            for i in range(3):
                linear(W[f"{blk}_o_w1"], gated_v[i], 128, 128, v_out[i])

        def _network():
            # block 1, block 2, final linear
            s_b1 = acts.tile([128, BC], f32, tag="s_mid", name="s_mid")
            v_b1 = [acts.tile([128, BC], f32, tag=f"v_mid{i}", name=f"v_mid{i}") for i in range(3)]
            block("b1", 64, sT, vT, s_b1, v_b1)
            s_b2 = acts.tile([128, BC], f32, tag="s_mid2", name="s_mid2")
            v_b2 = [acts.tile([128, BC], f32, tag=f"v_mid2{i}", name=f"v_mid2{i}") for i in range(3)]
            block("b2", 128, s_b1, v_b1, s_b2, v_b2)
            fo_a = acts.tile([128, BC], f32, tag="final_a", name="final_a")
            fo_b = acts.tile([128, BC], f32, tag="final_b", name="final_b")
            linear(W["f_w0"], s_b2, 128, 64, fo_a, out_rows=slice(0, 64))
            linear(W["f_w1"], v_b2[0], 128, 64, fo_a, out_rows=slice(64, 128))
            linear(W["f_w1"], v_b2[1], 128, 64, fo_b, out_rows=slice(0, 64))
            linear(W["f_w1"], v_b2[2], 128, 64, fo_b, out_rows=slice(64, 128))
            nc.sync.dma_start(out_d[0:128, :], fo_a[:])
            nc.sync.dma_start(out_d[128:256, :], fo_b[:])

        # repeat>1 wraps the network in an on-device loop (used only by
        # test.py for precise per-iteration timing; the grading path uses 1)
        if repeat > 1:
            with tc.For_i(0, repeat, 1):
                _network()
        else:
            _network()

    nc.finalize()
    return nc


def _host_prep(inputs):
    """Fold norm constants into weights; reorder/cast TP weights."""
    hf = np.float16
    d = {}
    for blk, M in (("b1", 64), ("b2", 128)):
        c_lin = np.float32(1.0 / np.sqrt(M))
        for nm in ("l1_w0", "l1_w1", "l2_w0", "l2_w1"):
            d[f"{blk}_{nm}"] = np.ascontiguousarray(
                inputs[f"{blk}_{nm}"] * c_lin, dtype=np.float32)
        c_tp = 1.0 / (M * np.sqrt(2.0))
        for nm, c in (("ss", c_tp), ("vv", c_tp / np.sqrt(3.0)),
                      ("sv", c_tp), ("vs", c_tp)):
            W = (inputs[f"{blk}_tp_{nm}"] * np.float32(c)).reshape(M * M, 128)
            # p-major: [128, K//128, 128]
            Wp = W.reshape(M * M // 128, 128, 128).transpose(1, 0, 2)
            d[f"{blk}_w_{nm}"] = np.ascontiguousarray(Wp).astype(hf)
        c_g = np.float32(1.0 / np.sqrt(128))
        for nm in ("g_ws", "g_wg", "g_wv"):
            d[f"{blk}_{nm}"] = np.ascontiguousarray(
                inputs[f"{blk}_{nm}"] * c_g, dtype=np.float32)
        c_og = np.float32(TANH_GAIN / np.sqrt(128))
        d[f"{blk}_o_w0"] = np.ascontiguousarray(
            inputs[f"{blk}_o_w0"] * c_og, dtype=np.float32)
        d[f"{blk}_o_w1"] = np.ascontiguousarray(
            inputs[f"{blk}_o_w1"] * c_og, dtype=np.float32)
    c_o = np.float32(1.0 / np.sqrt(128))
    d["f_w0"] = np.ascontiguousarray(inputs["f_w0"] * c_o, dtype=np.float32)
    d["f_w1"] = np.ascontiguousarray(inputs["f_w1"] * c_o, dtype=np.float32)
    d["idm"] = np.eye(128, dtype=np.float32).astype(hf)
    di = np.zeros((64, 4096), dtype=np.float32)
    for u in range(64):
        di[u, 64 * u:64 * u + 64] = 1.0
    d["di64"] = di.astype(hf)
    return d


def kernel(**inputs):
    from concourse.bass_utils import run_bass_kernel_spmd

    x = np.asarray(inputs["x"], dtype=np.float32)
    w = _host_prep({k: np.asarray(v, dtype=np.float32)
                    for k, v in inputs.items() if k != "x"})

    if "nc" not in _CACHE:
        _CACHE["nc"] = _build_program()
    nc = _CACHE["nc"]

    # shard + transpose to feature-major
    sT_full = np.ascontiguousarray(x[:, :64].T)                  # [64, B]
    v_full = x[:, 64:].reshape(B, 64, 3)
    vT_full = np.ascontiguousarray(v_full.transpose(2, 1, 0))    # [3, 64, B]
    in_maps = []
    for c in range(N_CORES):
        bs = slice(c * BC, (c + 1) * BC)
        m = dict(w)
        m["s0"] = np.ascontiguousarray(sT_full[:, bs])
        m["v0"] = np.ascontiguousarray(vT_full[:, :, bs]).reshape(192, BC)
        in_maps.append(m)

    res = run_bass_kernel_spmd(nc, in_maps, core_ids=list(range(N_CORES)))

    out = np.empty((B, 256), dtype=np.float32)
    for c in range(N_CORES):
        o = res.results[c]["out"]                                # [256, BC]
        bs = slice(c * BC, (c + 1) * BC)
        out[bs, :64] = o[:64].T
        # rows 64+64i+u = v comp i; ref layout col 64 + u*3 + i
        v = o[64:].reshape(3, 64, BC)
        out[bs, 64:] = v.transpose(2, 1, 0).reshape(BC, 192)
    return out



# revision 10
# speedup vs baseline: 1.8193x; 1.8193x over previous
"""Trainium2 Bass kernel for nn_EquivariantModel (e3nn-style equivariant net).

Strategy: data-parallel over batch (8 cores x 1024 rows), feature-major
activations.  All o3.Linear layers (l1/l2, block out-linears, final) are
folded host-side into the FullyConnectedTensorProduct weights, so each block
reduces to a bilinear form in its RAW inputs:

    tp_s[b,w] = sum_{pq} s_p s_q MSS[pq,w] + sum_{i,pq} v_ip v_iq MVV[pq,w]
    tp_v[b,w,i] = sum_{pq} s_p v_iq MX[pq,w]

The symmetric forms (s(x)s, v(x)v) need only cyclic diagonals d=0..M/2 (2x
fewer products); products z_d = x * rot_d(x) are built with single
full-width DVE multiplies against partition-rotated copies of x, which are
materialized by grouped DMA reads from a row-doubled DRAM image (one DMA
covers many rotations via an overlapping-stride access pattern).  Matmuls
stream z through per-diagonal weight tiles, accumulating in PSUM.  Block 1
(mult 64) stacks two batch halves in the partition dim and uses split-K
matmuls at base partitions 0/64.  A fraction of the products runs on the
GpSimd engine to unload the DVE.
"""

import sys
import numpy as np

if '/opt/trn_rl_repo' not in sys.path:
    sys.path.insert(0, '/opt/trn_rl_repo')

B, M_IN, M_HID = 8192, 64, 128
N_CORES = 8
BC = B // N_CORES            # batch per core
TANH_GAIN = 1.5927116870880127

GRP = 8                      # rotations / weight tiles per group DMA
GRPV = 4                     # v-rotation group size (3 streams alive at once)
POOL_EVERY = 8               # every Nth product goes to GpSimd instead of DVE

_CACHE = {}


def _build_program(repeat=1):
    import concourse.mybir as mybir
    import concourse.tile as tile
    from concourse import bacc
    from contextlib import ExitStack
    import bass_rust

    f16 = mybir.dt.float16
    f32 = mybir.dt.float32

    nc = bacc.Bacc("TRN2", target_bir_lowering=False)

    # ---- DRAM I/O ----
    s2d = nc.dram_tensor("s2d", [128, BC], f16, kind="ExternalInput")
    v2d = [nc.dram_tensor(f"v2d{i}", [128, BC], f16, kind="ExternalInput")
           for i in range(3)]
    wss1 = nc.dram_tensor("wss1", [128, 33, 128], f16, kind="ExternalInput")
    wvv1 = nc.dram_tensor("wvv1", [128, 33, 128], f16, kind="ExternalInput")
    wsv1 = nc.dram_tensor("wsv1", [128, 64, 128], f16, kind="ExternalInput")
    wss2 = nc.dram_tensor("wss2", [128, 65, 128], f16, kind="ExternalInput")
    wvv2 = nc.dram_tensor("wvv2", [128, 65, 128], f16, kind="ExternalInput")
    wsv2 = nc.dram_tensor("wsv2", [128, 128, 128], f16, kind="ExternalInput")
    gate_w = {}
    for blk in ("1", "2"):
        for nm in ("s", "g", "v"):
            gate_w[blk + nm] = nc.dram_tensor(f"g{blk}{nm}", [128, 128], f16,
                                              kind="ExternalInput")
    wfs = nc.dram_tensor("wfs", [128, 64], f16, kind="ExternalInput")
    wfv = nc.dram_tensor("wfv", [128, 64], f16, kind="ExternalInput")

    dts = nc.dram_tensor("dts", [256, BC], f16, kind="Internal")
    dgv = [nc.dram_tensor(f"dgv{i}", [256, BC], f16, kind="Internal")
           for i in range(3)]
    out_d = nc.dram_tensor("out", [256, BC], f32, kind="ExternalOutput")

    def src_ap(t, dims, offset):
        s = t[:].copy()
        s.ap = bass_rust.VecI64Pair(dims)
        s.offset = offset
        return s

    with ExitStack() as ctx:
        tc = ctx.enter_context(tile.TileContext(nc))
        consts = ctx.enter_context(tc.tile_pool(name="consts", bufs=1))
        acts = ctx.enter_context(tc.tile_pool(name="acts", bufs=1))
        rotp = ctx.enter_context(tc.tile_pool(name="rot", bufs=2))
        rotv = ctx.enter_context(tc.tile_pool(name="rotv", bufs=4))
        wstr = ctx.enter_context(tc.tile_pool(name="wstr", bufs=2))
        zp = ctx.enter_context(tc.tile_pool(name="z", bufs=8))
        psp = ctx.enter_context(tc.tile_pool(name="ps", bufs=1, space="PSUM"))
        tmp = ctx.enter_context(tc.tile_pool(name="tmp", bufs=1))

        GW = {}
        for k, t in gate_w.items():
            w = consts.tile([128, 128], f16, tag=f"gw{k}", name=f"gw{k}")
            nc.sync.dma_start(w[:], t[:])
            GW[k] = w
        wfs_sb = consts.tile([128, 64], f16, tag="wfs", name="wfs")
        nc.sync.dma_start(wfs_sb[:], wfs[:])
        wfv_sb = consts.tile([128, 64], f16, tag="wfv", name="wfv")
        nc.sync.dma_start(wfv_sb[:], wfv[:])

        nmul = [0]

        def mul(z, a, b):
            nmul[0] += 1
            if POOL_EVERY and nmul[0] % POOL_EVERY == 0:
                nc.gpsimd.tensor_mul(z, a, b)
            else:
                nc.vector.tensor_mul(z, a, b)

        def b1_block():
            # bases (stacked halves: rows 0-63 = batch 0:512, 64-127 = 512:1024)
            sb = acts.tile([128, 512], f16, tag="sb1", name="sb1")
            nc.sync.dma_start(sb[0:64, :], s2d[0:64, 0:512])
            nc.sync.dma_start(sb[64:128, :], s2d[0:64, 512:1024])
            vb = []
            for i in range(3):
                t = acts.tile([128, 512], f16, tag=f"vb1{i}", name=f"vb1{i}")
                nc.sync.dma_start(t[0:64, :], v2d[i][0:64, 0:512])
                nc.sync.dma_start(t[64:128, :], v2d[i][0:64, 512:1024])
                vb.append(t)

            accs = psp.tile([128, 1024], f32, tag="pa_s", name="pa_s")
            accv = [psp.tile([128, 1024], f32, tag=f"pa_v{i}", name=f"pa_v{i}")
                    for i in range(3)]

            def mm2(acc, w, z, start, stop):
                nc.tensor.matmul(acc[:, 0:512], w[0:64, :], z[0:64, :],
                                 start=start, stop=stop, tile_position=(0, 0))
                nc.tensor.matmul(acc[:, 512:1024], w[64:128, :], z[64:128, :],
                                 start=start, stop=stop, tile_position=(64, 0))

            # phase 1: s-rotations d=0..63 -> sv' (all d) + ss (d<=32)
            for g0 in range(0, 64, GRP):
                ng = min(GRP, 64 - g0)
                rot = rotp.tile([128, GRP * 512], f16, tag="rotS1", name="rotS1")
                for h in range(2):
                    nc.sync.dma_start(
                        rot[h * 64:(h + 1) * 64, 0:ng * 512],
                        src_ap(s2d, [[BC, 64], [BC, ng], [1, 512]],
                               g0 * BC + h * 512))
                wsv_t = wstr.tile([128, GRP * 128], f16, tag="wsv", name="wsv")
                nc.scalar.dma_start(
                    wsv_t[:, 0:ng * 128],
                    wsv1[:, g0:g0 + ng, :].rearrange("p n m -> p (n m)"))
                nss = max(0, min(ng, 33 - g0))
                if nss > 0:
                    wss_t = wstr.tile([128, GRP * 128], f16, tag="wss", name="wss")
                    nc.scalar.dma_start(
                        wss_t[:, 0:nss * 128],
                        wss1[:, g0:g0 + nss, :].rearrange("p n m -> p (n m)"))
                for j in range(ng):
                    d = g0 + j
                    rj = rot[:, j * 512:(j + 1) * 512]
                    for i in range(3):
                        z = zp.tile([128, 512], f16, tag="z1", name="z1")
                        mul(z, rj, vb[i])
                        mm2(accv[i], wsv_t[:, j * 128:(j + 1) * 128], z,
                            start=(d == 0), stop=(d == 63))
                    if d <= 32:
                        z = zp.tile([128, 512], f16, tag="z1", name="z1")
                        mul(z, sb, rj)
                        mm2(accs, wss_t[:, j * 128:(j + 1) * 128], z,
                            start=(d == 0), stop=False)
            # phase 2: v-rotations d=0..32 -> vv into acc_s
            for g0 in range(0, 33, GRPV):
                ng = min(GRPV, 33 - g0)
                rots = []
                for i in range(3):
                    rot = rotv.tile([128, GRPV * 512], f16, tag="rotV1",
                                    name="rotV1")
                    for h in range(2):
                        nc.sync.dma_start(
                            rot[h * 64:(h + 1) * 64, 0:ng * 512],
                            src_ap(v2d[i], [[BC, 64], [BC, ng], [1, 512]],
                                   g0 * BC + h * 512))
                    rots.append(rot)
                wvv_t = wstr.tile([128, GRP * 128], f16, tag="wvv", name="wvv")
                nc.scalar.dma_start(
                    wvv_t[:, 0:ng * 128],
                    wvv1[:, g0:g0 + ng, :].rearrange("p n m -> p (n m)"))
                for j in range(ng):
                    d = g0 + j
                    for i in range(3):
                        z = zp.tile([128, 512], f16, tag="z1", name="z1")
                        mul(z, vb[i], rots[i][:, j * 512:(j + 1) * 512])
                        mm2(accs, wvv_t[:, j * 128:(j + 1) * 128], z,
                            start=False, stop=(d == 32 and i == 2))
            return accs, accv

        def b2_block(tanh_s, gated):
            accs = psp.tile([128, 1024], f32, tag="pa_s", name="pa_s")
            accv = [psp.tile([128, 1024], f32, tag=f"pa_v{i}", name=f"pa_v{i}")
                    for i in range(3)]

            def mm2(acc, w, z, start, stop):
                nc.tensor.matmul(acc[:, 0:512], w, z[:, 0:512],
                                 start=start, stop=stop)
                nc.tensor.matmul(acc[:, 512:1024], w, z[:, 512:1024],
                                 start=start, stop=stop)

            # phase 1: s-rotations d=0..127 -> sv' (all d) + ss (d<=64)
            for g0 in range(0, 128, GRP):
                ng = min(GRP, 128 - g0)
                rot = rotp.tile([128, GRP * 1024], f16, tag="rotS2", name="rotS2")
                nc.sync.dma_start(
                    rot[:, 0:ng * 1024],
                    src_ap(dts, [[BC, 128], [BC, ng], [1, 1024]], g0 * BC))
                wsv_t = wstr.tile([128, GRP * 128], f16, tag="wsv", name="wsv")
                nc.scalar.dma_start(
                    wsv_t[:, 0:ng * 128],
                    wsv2[:, g0:g0 + ng, :].rearrange("p n m -> p (n m)"))
                nss = max(0, min(ng, 65 - g0))
                if nss > 0:
                    wss_t = wstr.tile([128, GRP * 128], f16, tag="wss", name="wss")
                    nc.scalar.dma_start(
                        wss_t[:, 0:nss * 128],
                        wss2[:, g0:g0 + nss, :].rearrange("p n m -> p (n m)"))
                for j in range(ng):
                    d = g0 + j
                    rj = rot[:, j * 1024:(j + 1) * 1024]
                    for i in range(3):
                        z = zp.tile([128, 1024], f16, tag="z2", name="z2")
                        mul(z, rj, gated[i])
                        mm2(accv[i], wsv_t[:, j * 128:(j + 1) * 128], z,
                            start=(d == 0), stop=(d == 127))
                    if d <= 64:
                        z = zp.tile([128, 1024], f16, tag="z2", name="z2")
                        mul(z, tanh_s, rj)
                        mm2(accs, wss_t[:, j * 128:(j + 1) * 128], z,
                            start=(d == 0), stop=False)
            # phase 2: v-rotations d=0..64 -> vv into acc_s
            for g0 in range(0, 65, GRPV):
                ng = min(GRPV, 65 - g0)
                rots = []
                for i in range(3):
                    rot = rotv.tile([128, GRPV * 1024], f16, tag="rotV2",
                                    name="rotV2")
                    nc.sync.dma_start(
                        rot[:, 0:ng * 1024],
                        src_ap(dgv[i], [[BC, 128], [BC, ng], [1, 1024]], g0 * BC))
                    rots.append(rot)
                wvv_t = wstr.tile([128, GRP * 128], f16, tag="wvv", name="wvv")
                nc.scalar.dma_start(
                    wvv_t[:, 0:ng * 128],
                    wvv2[:, g0:g0 + ng, :].rearrange("p n m -> p (n m)"))
                for j in range(ng):
                    d = g0 + j
                    for i in range(3):
                        z = zp.tile([128, 1024], f16, tag="z2", name="z2")
                        mul(z, gated[i], rots[i][:, j * 1024:(j + 1) * 1024])
                        mm2(accs, wvv_t[:, j * 128:(j + 1) * 128], z,
                            start=False, stop=(d == 64 and i == 2))
            return accs, accv

        def gate(blk, accs, accv):
            """PSUM accs -> (tanh_s, gated_v[3]) f16 [128, BC]."""
            tp_s = acts.tile([128, BC], f16, tag="tps", name="tps")
            nc.scalar.copy(tp_s[:, :], accs[:, :])
            tp_v = []
            for i in range(3):
                t = acts.tile([128, BC], f16, tag=f"tpv{i}", name=f"tpv{i}")
                nc.scalar.copy(t[:, :], accv[i][:, :])
                tp_v.append(t)
            tanh_s = acts.tile([128, BC], f16, tag=f"ths{blk}", name=f"ths{blk}")
            tg = acts.tile([128, BC], f16, tag="tg", name="tg")
            vl = [acts.tile([128, BC], f16, tag=f"vl{i}", name=f"vl{i}")
                  for i in range(3)]
            # gate matmuls reuse the freed accumulator PSUM slots
            ps = psp.tile([128, 1024], f32, tag="pa_s", name="pa_s")
            psg = psp.tile([128, 1024], f32, tag="pa_v0", name="pa_v0")
            psv = [psp.tile([128, 1024], f32, tag=f"pa_v{i}", name=f"pa_v{i}")
                   for i in (1, 2)]
            psv.append(psp.tile([128, 1024], f32, tag="pa_s", name="pa_s"))
            for h in range(2):
                sl_ = slice(h * 512, (h + 1) * 512)
                nc.tensor.matmul(ps[:, sl_], GW[blk + "s"], tp_s[:, sl_],
                                 start=True, stop=True)
                nc.scalar.activation(tanh_s[:, sl_], ps[:, sl_],
                                     mybir.ActivationFunctionType.Tanh)
                nc.tensor.matmul(psg[:, sl_], GW[blk + "g"], tp_s[:, sl_],
                                 start=True, stop=True)
                nc.scalar.activation(tg[:, sl_], psg[:, sl_],
                                     mybir.ActivationFunctionType.Tanh)
                for i in range(3):
                    nc.tensor.matmul(psv[i][:, sl_], GW[blk + "v"],
                                     tp_v[i][:, sl_], start=True, stop=True)
                    nc.scalar.copy(vl[i][:, sl_], psv[i][:, sl_])
            gated = []
            for i in range(3):
                t = acts.tile([128, BC], f16, tag=f"gv{blk}{i}", name=f"gv{blk}{i}")
                nc.vector.tensor_mul(t, tg, vl[i])
                gated.append(t)
            return tanh_s, gated

        def _network():
            accs, accv = b1_block()
            tanh_s1, gated1 = gate("1", accs, accv)
            # doubled DRAM images for b2 rotations
            nc.sync.dma_start(dts[0:128, :], tanh_s1[:])
            nc.sync.dma_start(dts[128:256, :], tanh_s1[:])
            for i in range(3):
                nc.sync.dma_start(dgv[i][0:128, :], gated1[i][:])
                nc.sync.dma_start(dgv[i][128:256, :], gated1[i][:])
            accs2, accv2 = b2_block(tanh_s1, gated1)
            tanh_s2, gated2 = gate("2", accs2, accv2)
            # final linears (out-linears folded in)
            outs = tmp.tile([64, BC], f32, tag="outs", name="outs")
            outv = [tmp.tile([64, BC], f32, tag=f"outv{i}", name=f"outv{i}")
                    for i in range(3)]
            fps = psp.tile([128, 1024], f32, tag="pa_v0", name="pa_v0")
            fpv = [psp.tile([128, 1024], f32, tag=t, name=t)
                   for t in ("pa_v1", "pa_v2", "pa_s")]
            for h in range(2):
                sl_ = slice(h * 512, (h + 1) * 512)
                nc.tensor.matmul(fps[0:64, sl_], wfs_sb[:], tanh_s2[:, sl_],
                                 start=True, stop=True)
                nc.scalar.copy(outs[:, sl_], fps[0:64, sl_])
                for i in range(3):
                    nc.tensor.matmul(fpv[i][0:64, sl_], wfv_sb[:],
                                     gated2[i][:, sl_], start=True, stop=True)
                    nc.scalar.copy(outv[i][:, sl_], fpv[i][0:64, sl_])
            nc.sync.dma_start(out_d[0:64, :], outs[:])
            for i in range(3):
                nc.sync.dma_start(out_d[64 + 64 * i:128 + 64 * i, :], outv[i][:])

        if repeat > 1:
            with tc.For_i(0, repeat, 1):
                _network()
        else:
            _network()

    nc.finalize()
    return nc


def _host_prep(inputs):
    """Fold all linears into TP weights; build diagonal weight streams."""
    f = {k: np.asarray(v, np.float64) for k, v in inputs.items() if k != 'x'}
    d = {}
    fold = {}
    for blk, M in (("b1", 64), ("b2", 128)):
        c1 = 1.0 / np.sqrt(M)
        A, Av = f[f"{blk}_l1_w0"] * c1, f[f"{blk}_l1_w1"] * c1
        Bm, Bv = f[f"{blk}_l2_w0"] * c1, f[f"{blk}_l2_w1"] * c1
        if blk == "b2":
            A, Av = fold["O1s"] @ A, fold["O1v"] @ Av
            Bm, Bv = fold["O1s"] @ Bm, fold["O1v"] @ Bv
        ctp = 1.0 / (M * np.sqrt(2.0))

        def fld(L, R, W, c):
            T = np.tensordot(L, W, axes=(1, 0))
            T = np.tensordot(R, T, axes=(1, 1))
            return c * T.transpose(1, 0, 2)
        MSS = fld(A, Bm, f[f"{blk}_tp_ss"], ctp)
        MVV = fld(Av, Bv, f[f"{blk}_tp_vv"], ctp / np.sqrt(3.0))
        MSV = fld(A, Bv, f[f"{blk}_tp_sv"], ctp)
        MVS = fld(Av, Bm, f[f"{blk}_tp_vs"], ctp)
        MX = MSV + MVS.transpose(1, 0, 2)
        Min = A.shape[0]
        ar = np.arange(Min)
        sym = {}
        for nm, Msym in (("ss", MSS), ("vv", MVV)):
            tiles = []
            for dd in range(Min // 2 + 1):
                idx = (ar + dd) % Min
                if dd == 0:
                    w = Msym[ar, ar, :]
                elif dd == Min // 2:
                    w = (Msym[ar, idx, :] + Msym[idx, ar, :]) * 0.5
                else:
                    w = Msym[ar, idx, :] + Msym[idx, ar, :]
                tiles.append(w)
            sym[nm] = np.stack(tiles, axis=1)                 # [Min, nd, 128]
        rect = np.stack([MX[(ar + c) % Min, ar, :] for c in range(Min)],
                        axis=1)                               # [Min, Min, 128]
        if blk == "b1":
            for nm in ("ss", "vv"):
                sym[nm] = np.concatenate([sym[nm], sym[nm]], axis=0)
            rect = np.concatenate([rect, rect], axis=0)
        bn = blk[1]
        d[f"wss{bn}"] = np.ascontiguousarray(sym["ss"]).astype(np.float16)
        d[f"wvv{bn}"] = np.ascontiguousarray(sym["vv"]).astype(np.float16)
        d[f"wsv{bn}"] = np.ascontiguousarray(rect).astype(np.float16)
        cg = 1.0 / np.sqrt(128)
        for nm, sfx in (("ws", "s"), ("wg", "g"), ("wv", "v")):
            d[f"g{bn}{sfx}"] = (f[f"{blk}_g_{nm}"] * cg).astype(np.float16)
        cog = TANH_GAIN / np.sqrt(128)
        fold[f"O{bn}s"] = f[f"{blk}_o_w0"] * cog
        fold[f"O{bn}v"] = f[f"{blk}_o_w1"] * cog
    cf = 1.0 / np.sqrt(128)
    d["wfs"] = (fold["O2s"] @ (f["f_w0"] * cf)).astype(np.float16)
    d["wfv"] = (fold["O2v"] @ (f["f_w1"] * cf)).astype(np.float16)
    return d


def _make_in_maps(x, w):
    x = np.asarray(x, dtype=np.float32)
    in_maps = []
    for c in range(N_CORES):
        bs = slice(c * BC, (c + 1) * BC)
        xl = x[bs]
        s_loc = np.ascontiguousarray(xl[:, :64].T).astype(np.float16)   # [64, BC]
        v_loc = xl[:, 64:].reshape(BC, 64, 3)
        m = dict(w)
        m["s2d"] = np.concatenate([s_loc, s_loc], axis=0)
        for i in range(3):
            vi = np.ascontiguousarray(v_loc[:, :, i].T).astype(np.float16)
            m[f"v2d{i}"] = np.concatenate([vi, vi], axis=0)
        in_maps.append(m)
    return in_maps


def kernel(**inputs):
    from concourse.bass_utils import run_bass_kernel_spmd

    w = _host_prep(inputs)
    in_maps = _make_in_maps(inputs["x"], w)

    if "nc" not in _CACHE:
        _CACHE["nc"] = _build_program()
    nc = _CACHE["nc"]

    res = run_bass_kernel_spmd(nc, in_maps, core_ids=list(range(N_CORES)))

    out = np.empty((B, 256), dtype=np.float32)
    for c in range(N_CORES):
        o = res.results[c]["out"]                                # [256, BC]
        bs = slice(c * BC, (c + 1) * BC)
        out[bs, :64] = o[:64].T
        v = o[64:].reshape(3, 64, BC)
        out[bs, 64:] = v.transpose(2, 1, 0).reshape(BC, 192)
    return out


# revision 12
# speedup vs baseline: 1.8765x; 1.0314x over previous
"""Trainium2 Bass kernel for nn_EquivariantModel (e3nn-style equivariant net).

Strategy: data-parallel over batch (8 cores x 1024 rows), feature-major
activations.  All o3.Linear layers (l1/l2, block out-linears, final) are
folded host-side into the FullyConnectedTensorProduct weights, so each block
reduces to a bilinear form in its RAW inputs:

    tp_s[b,w] = sum_{pq} s_p s_q MSS[pq,w] + sum_{i,pq} v_ip v_iq MVV[pq,w]
    tp_v[b,w,i] = sum_{pq} s_p v_iq MX[pq,w]

The symmetric forms (s(x)s, v(x)v) need only cyclic diagonals d=0..M/2 (2x
fewer products); products z_d = x * rot_d(x) are built with single
full-width DVE multiplies against partition-rotated copies of x, which are
materialized by grouped DMA reads from a row-doubled DRAM image (one DMA
covers many rotations via an overlapping-stride access pattern; the three
vector components share one image so a group is a single DMA).  Matmuls
stream z through per-diagonal weight tiles, accumulating in PSUM; the ss,
vv and sv streams are interleaved in one d-loop to keep DMA demand flat.
Block 1 (mult 64) stacks two batch halves in the partition dim and uses
split-K matmuls at base partitions 0/64.  A fraction of the products runs
on the GpSimd engine to unload the DVE.
"""

import sys
import numpy as np

if '/opt/trn_rl_repo' not in sys.path:
    sys.path.insert(0, '/opt/trn_rl_repo')

B, M_IN, M_HID = 8192, 64, 128
N_CORES = 8
BC = B // N_CORES            # batch per core
TANH_GAIN = 1.5927116870880127

GRP = 8                      # s-rotations / weight tiles per group DMA
GRPV = 2                     # v-rotation group size (3 components per group)
POOL_EVERY = 5               # every Nth product goes to GpSimd instead of DVE

_CACHE = {}


def _build_program(repeat=1):
    import concourse.mybir as mybir
    import concourse.tile as tile
    from concourse import bacc
    from contextlib import ExitStack
    import bass_rust

    f16 = mybir.dt.float16
    f32 = mybir.dt.float32

    nc = bacc.Bacc("TRN2", target_bir_lowering=False)

    # ---- DRAM I/O ----
    s2d = nc.dram_tensor("s2d", [128, BC], f16, kind="ExternalInput")
    v2da = nc.dram_tensor("v2da", [128, 3 * BC], f16, kind="ExternalInput")
    wss1 = nc.dram_tensor("wss1", [128, 33, 128], f16, kind="ExternalInput")
    wvv1 = nc.dram_tensor("wvv1", [128, 33, 128], f16, kind="ExternalInput")
    wsv1 = nc.dram_tensor("wsv1", [128, 64, 128], f16, kind="ExternalInput")
    wss2 = nc.dram_tensor("wss2", [128, 65, 128], f16, kind="ExternalInput")
    wvv2 = nc.dram_tensor("wvv2", [128, 65, 128], f16, kind="ExternalInput")
    wsv2 = nc.dram_tensor("wsv2", [128, 128, 128], f16, kind="ExternalInput")
    gate_w = {}
    for blk in ("1", "2"):
        for nm in ("s", "g", "v"):
            gate_w[blk + nm] = nc.dram_tensor(f"g{blk}{nm}", [128, 128], f16,
                                              kind="ExternalInput")
    wfs = nc.dram_tensor("wfs", [128, 64], f16, kind="ExternalInput")
    wfv = nc.dram_tensor("wfv", [128, 64], f16, kind="ExternalInput")

    dts = nc.dram_tensor("dts", [256, BC], f16, kind="Internal")
    dgva = nc.dram_tensor("dgva", [256, 3 * BC], f16, kind="Internal")
    out_d = nc.dram_tensor("out", [256, BC], f32, kind="ExternalOutput")

    def src_ap(t, dims, offset):
        s = t[:].copy()
        s.ap = bass_rust.VecI64Pair(dims)
        s.offset = offset
        return s

    with ExitStack() as ctx:
        tc = ctx.enter_context(tile.TileContext(nc))
        consts = ctx.enter_context(tc.tile_pool(name="consts", bufs=1))
        acts = ctx.enter_context(tc.tile_pool(name="acts", bufs=1))
        rotp = ctx.enter_context(tc.tile_pool(name="rot", bufs=2))
        rotv = ctx.enter_context(tc.tile_pool(name="rotv", bufs=3))
        wstr = ctx.enter_context(tc.tile_pool(name="wstr", bufs=2))
        wstv = ctx.enter_context(tc.tile_pool(name="wstv", bufs=3))
        zp = ctx.enter_context(tc.tile_pool(name="z", bufs=8))
        psp = ctx.enter_context(tc.tile_pool(name="ps", bufs=1, space="PSUM"))
        tmp = ctx.enter_context(tc.tile_pool(name="tmp", bufs=1))

        GW = {}
        for k, t in gate_w.items():
            w = consts.tile([128, 128], f16, tag=f"gw{k}", name=f"gw{k}")
            nc.sync.dma_start(w[:], t[:])
            GW[k] = w
        wfs_sb = consts.tile([128, 64], f16, tag="wfs", name="wfs")
        nc.sync.dma_start(wfs_sb[:], wfs[:])
        wfv_sb = consts.tile([128, 64], f16, tag="wfv", name="wfv")
        nc.sync.dma_start(wfv_sb[:], wfv[:])

        nmul = [0]

        def mul(z, a, b):
            nmul[0] += 1
            if POOL_EVERY and nmul[0] % POOL_EVERY == 0:
                nc.gpsimd.tensor_mul(z, a, b)
            else:
                nc.vector.tensor_mul(z, a, b)

        def b1_block():
            # bases (stacked halves: rows 0-63 = batch 0:512, 64-127 = 512:1024)
            sb = acts.tile([128, 512], f16, tag="sb1", name="sb1")
            nc.sync.dma_start(sb[0:64, :], s2d[0:64, 0:512])
            nc.sync.dma_start(sb[64:128, :], s2d[0:64, 512:1024])
            vb = []
            for i in range(3):
                t = acts.tile([128, 512], f16, tag=f"vb1{i}", name=f"vb1{i}")
                nc.sync.dma_start(t[0:64, :], v2da[0:64, i * BC:i * BC + 512])
                nc.sync.dma_start(t[64:128, :],
                                  v2da[0:64, i * BC + 512:(i + 1) * BC])
                vb.append(t)

            accs = psp.tile([128, 1024], f32, tag="pa_s", name="pa_s")
            accv = [psp.tile([128, 1024], f32, tag=f"pa_v{i}", name=f"pa_v{i}")
                    for i in range(3)]

            def mm2(acc, w, z, start, stop):
                nc.tensor.matmul(acc[:, 0:512], w[0:64, :], z[0:64, :],
                                 start=start, stop=stop, tile_position=(0, 0))
                nc.tensor.matmul(acc[:, 512:1024], w[64:128, :], z[64:128, :],
                                 start=start, stop=stop, tile_position=(64, 0))

            vrot = [None]
            wvv_t = [None]

            for g0 in range(0, 64, GRP):
                ng = min(GRP, 64 - g0)
                rot = rotp.tile([128, GRP * 512], f16, tag="rotS1", name="rotS1")
                for h in range(2):
                    nc.sync.dma_start(
                        rot[h * 64:(h + 1) * 64, 0:ng * 512],
                        src_ap(s2d, [[BC, 64], [BC, ng], [1, 512]],
                               g0 * BC + h * 512))
                wsv_t = wstr.tile([128, GRP * 128], f16, tag="wsv", name="wsv")
                nc.scalar.dma_start(
                    wsv_t[:, 0:ng * 128],
                    wsv1[:, g0:g0 + ng, :].rearrange("p n m -> p (n m)"))
                nss = max(0, min(ng, 33 - g0))
                if nss > 0:
                    wss_t = wstr.tile([128, GRP * 128], f16, tag="wss", name="wss")
                    nc.scalar.dma_start(
                        wss_t[:, 0:nss * 128],
                        wss1[:, g0:g0 + nss, :].rearrange("p n m -> p (n m)"))
                for j in range(ng):
                    d = g0 + j
                    rj = rot[:, j * 512:(j + 1) * 512]
                    for i in range(3):
                        z = zp.tile([128, 512], f16, tag="z1", name="z1")
                        mul(z, rj, vb[i])
                        mm2(accv[i], wsv_t[:, j * 128:(j + 1) * 128], z,
                            start=(d == 0), stop=(d == 63))
                    if d <= 32:
                        z = zp.tile([128, 512], f16, tag="z1", name="z1")
                        mul(z, sb, rj)
                        mm2(accs, wss_t[:, j * 128:(j + 1) * 128], z,
                            start=(d == 0), stop=False)
                        # interleaved vv stream (combined 3-component groups)
                        if d % GRPV == 0:
                            nv = min(GRPV, 33 - d)
                            vr = rotv.tile([128, GRPV * 3 * 512], f16,
                                           tag="rotV1", name="rotV1")
                            for h in range(2):
                                nc.sync.dma_start(
                                    vr[h * 64:(h + 1) * 64, 0:nv * 3 * 512],
                                    src_ap(v2da,
                                           [[3 * BC, 64], [3 * BC, nv],
                                            [BC, 3], [1, 512]],
                                           d * 3 * BC + h * 512))
                            vrot[0] = vr
                            wt = wstv.tile([128, GRPV * 128], f16, tag="wvv",
                                           name="wvv")
                            nc.scalar.dma_start(
                                wt[:, 0:nv * 128],
                                wvv1[:, d:d + nv, :].rearrange(
                                    "p n m -> p (n m)"))
                            wvv_t[0] = wt
                        jv = d % GRPV
                        for i in range(3):
                            z = zp.tile([128, 512], f16, tag="z1", name="z1")
                            mul(z, vb[i],
                                vrot[0][:, (jv * 3 + i) * 512:
                                         (jv * 3 + i + 1) * 512])
                            mm2(accs, wvv_t[0][:, jv * 128:(jv + 1) * 128], z,
                                start=False, stop=(d == 32 and i == 2))
            return accs, accv

        def b2_block(tanh_s, gated):
            accs = psp.tile([128, 1024], f32, tag="pa_s", name="pa_s")
            accv = [psp.tile([128, 1024], f32, tag=f"pa_v{i}", name=f"pa_v{i}")
                    for i in range(3)]

            def mm2(acc, w, z, start, stop):
                nc.tensor.matmul(acc[:, 0:512], w, z[:, 0:512],
                                 start=start, stop=stop)
                nc.tensor.matmul(acc[:, 512:1024], w, z[:, 512:1024],
                                 start=start, stop=stop)

            vrot = [None]
            wvv_t = [None]

            for g0 in range(0, 128, GRP):
                ng = min(GRP, 128 - g0)
                rot = rotp.tile([128, GRP * 1024], f16, tag="rotS2", name="rotS2")
                nc.sync.dma_start(
                    rot[:, 0:ng * 1024],
                    src_ap(dts, [[BC, 128], [BC, ng], [1, 1024]], g0 * BC))
                wsv_t = wstr.tile([128, GRP * 128], f16, tag="wsv", name="wsv")
                nc.scalar.dma_start(
                    wsv_t[:, 0:ng * 128],
                    wsv2[:, g0:g0 + ng, :].rearrange("p n m -> p (n m)"))
                nss = max(0, min(ng, 65 - g0))
                if nss > 0:
                    wss_t = wstr.tile([128, GRP * 128], f16, tag="wss", name="wss")
                    nc.scalar.dma_start(
                        wss_t[:, 0:nss * 128],
                        wss2[:, g0:g0 + nss, :].rearrange("p n m -> p (n m)"))
                for j in range(ng):
                    d = g0 + j
                    rj = rot[:, j * 1024:(j + 1) * 1024]
                    for i in range(3):
                        z = zp.tile([128, 1024], f16, tag="z2", name="z2")
                        mul(z, rj, gated[i])
                        mm2(accv[i], wsv_t[:, j * 128:(j + 1) * 128], z,
                            start=(d == 0), stop=(d == 127))
                    if d <= 64:
                        z = zp.tile([128, 1024], f16, tag="z2", name="z2")
                        mul(z, tanh_s, rj)
                        mm2(accs, wss_t[:, j * 128:(j + 1) * 128], z,
                            start=(d == 0), stop=False)
                        if d % GRPV == 0:
                            nv = min(GRPV, 65 - d)
                            vr = rotv.tile([128, GRPV * 3 * 1024], f16,
                                           tag="rotV2", name="rotV2")
                            nc.sync.dma_start(
                                vr[:, 0:nv * 3 * 1024],
                                src_ap(dgva,
                                       [[3 * BC, 128], [3 * BC, nv],
                                        [1, 3 * BC]],
                                       d * 3 * BC))
                            vrot[0] = vr
                            wt = wstv.tile([128, GRPV * 128], f16, tag="wvv",
                                           name="wvv")
                            nc.scalar.dma_start(
                                wt[:, 0:nv * 128],
                                wvv2[:, d:d + nv, :].rearrange(
                                    "p n m -> p (n m)"))
                            wvv_t[0] = wt
                        jv = d % GRPV
                        for i in range(3):
                            z = zp.tile([128, 1024], f16, tag="z2", name="z2")
                            mul(z, gated[i],
                                vrot[0][:, (jv * 3 + i) * 1024:
                                        (jv * 3 + i + 1) * 1024])
                            mm2(accs, wvv_t[0][:, jv * 128:(jv + 1) * 128], z,
                                start=False, stop=(d == 64 and i == 2))
            return accs, accv

        def gate(blk, accs, accv):
            """PSUM accs -> (tanh_s, gated_v[3]) f16 [128, BC]."""
            tp_s = acts.tile([128, BC], f16, tag="tps", name="tps")
            nc.scalar.copy(tp_s[:, :], accs[:, :])
            tp_v = []
            for i in range(3):
                t = acts.tile([128, BC], f16, tag=f"tpv{i}", name=f"tpv{i}")
                nc.scalar.copy(t[:, :], accv[i][:, :])
                tp_v.append(t)
            tanh_s = acts.tile([128, BC], f16, tag=f"ths{blk}", name=f"ths{blk}")
            tg = acts.tile([128, BC], f16, tag="tg", name="tg")
            vl = [acts.tile([128, BC], f16, tag=f"vl{i}", name=f"vl{i}")
                  for i in range(3)]
            # gate matmuls reuse the freed accumulator PSUM slots
            ps = psp.tile([128, 1024], f32, tag="pa_s", name="pa_s")
            psg = psp.tile([128, 1024], f32, tag="pa_v0", name="pa_v0")
            psv = [psp.tile([128, 1024], f32, tag=f"pa_v{i}", name=f"pa_v{i}")
                   for i in (1, 2)]
            psv.append(psp.tile([128, 1024], f32, tag="pa_s", name="pa_s"))
            for h in range(2):
                sl_ = slice(h * 512, (h + 1) * 512)
                nc.tensor.matmul(ps[:, sl_], GW[blk + "s"], tp_s[:, sl_],
                                 start=True, stop=True)
                nc.scalar.activation(tanh_s[:, sl_], ps[:, sl_],
                                     mybir.ActivationFunctionType.Tanh)
                nc.tensor.matmul(psg[:, sl_], GW[blk + "g"], tp_s[:, sl_],
                                 start=True, stop=True)
                nc.scalar.activation(tg[:, sl_], psg[:, sl_],
                                     mybir.ActivationFunctionType.Tanh)
                for i in range(3):
                    nc.tensor.matmul(psv[i][:, sl_], GW[blk + "v"],
                                     tp_v[i][:, sl_], start=True, stop=True)
                    nc.scalar.copy(vl[i][:, sl_], psv[i][:, sl_])
            gated = []
            for i in range(3):
                t = acts.tile([128, BC], f16, tag=f"gv{blk}{i}", name=f"gv{blk}{i}")
                nc.vector.tensor_mul(t, tg, vl[i])
                gated.append(t)
            return tanh_s, gated

        def _network():
            accs, accv = b1_block()
            tanh_s1, gated1 = gate("1", accs, accv)
            # doubled DRAM images for b2 rotations
            nc.sync.dma_start(dts[0:128, :], tanh_s1[:])
            nc.sync.dma_start(dts[128:256, :], tanh_s1[:])
            for i in range(3):
                nc.sync.dma_start(dgva[0:128, i * BC:(i + 1) * BC], gated1[i][:])
                nc.sync.dma_start(dgva[128:256, i * BC:(i + 1) * BC], gated1[i][:])
            accs2, accv2 = b2_block(tanh_s1, gated1)
            tanh_s2, gated2 = gate("2", accs2, accv2)
            # final linears (out-linears folded in)
            outs = tmp.tile([64, BC], f32, tag="outs", name="outs")
            outv = [tmp.tile([64, BC], f32, tag=f"outv{i}", name=f"outv{i}")
                    for i in range(3)]
            fps = psp.tile([128, 1024], f32, tag="pa_v0", name="pa_v0")
            fpv = [psp.tile([128, 1024], f32, tag=t, name=t)
                   for t in ("pa_v1", "pa_v2", "pa_s")]
            for h in range(2):
                sl_ = slice(h * 512, (h + 1) * 512)
                nc.tensor.matmul(fps[0:64, sl_], wfs_sb[:], tanh_s2[:, sl_],
                                 start=True, stop=True)
                nc.scalar.copy(outs[:, sl_], fps[0:64, sl_])
                for i in range(3):
                    nc.tensor.matmul(fpv[i][0:64, sl_], wfv_sb[:],
                                     gated2[i][:, sl_], start=True, stop=True)
                    nc.scalar.copy(outv[i][:, sl_], fpv[i][0:64, sl_])
            nc.sync.dma_start(out_d[0:64, :], outs[:])
            for i in range(3):
                nc.sync.dma_start(out_d[64 + 64 * i:128 + 64 * i, :], outv[i][:])

        if repeat > 1:
            with tc.For_i(0, repeat, 1):
                _network()
        else:
            _network()

    nc.finalize()
    return nc


def _host_prep(inputs):
    """Fold all linears into TP weights; build diagonal weight streams."""
    f = {k: np.asarray(v, np.float64) for k, v in inputs.items() if k != 'x'}
    d = {}
    fold = {}
    for blk, M in (("b1", 64), ("b2", 128)):
        c1 = 1.0 / np.sqrt(M)
        A, Av = f[f"{blk}_l1_w0"] * c1, f[f"{blk}_l1_w1"] * c1
        Bm, Bv = f[f"{blk}_l2_w0"] * c1, f[f"{blk}_l2_w1"] * c1
        if blk == "b2":
            A, Av = fold["O1s"] @ A, fold["O1v"] @ Av
            Bm, Bv = fold["O1s"] @ Bm, fold["O1v"] @ Bv
        ctp = 1.0 / (M * np.sqrt(2.0))

        def fld(L, R, W, c):
            T = np.tensordot(L, W, axes=(1, 0))
            T = np.tensordot(R, T, axes=(1, 1))
            return c * T.transpose(1, 0, 2)
        MSS = fld(A, Bm, f[f"{blk}_tp_ss"], ctp)
        MVV = fld(Av, Bv, f[f"{blk}_tp_vv"], ctp / np.sqrt(3.0))
        MSV = fld(A, Bv, f[f"{blk}_tp_sv"], ctp)
        MVS = fld(Av, Bm, f[f"{blk}_tp_vs"], ctp)
        MX = MSV + MVS.transpose(1, 0, 2)
        Min = A.shape[0]
        ar = np.arange(Min)
        sym = {}
        for nm, Msym in (("ss", MSS), ("vv", MVV)):
            tiles = []
            for dd in range(Min // 2 + 1):
                idx = (ar + dd) % Min
                if dd == 0:
                    w = Msym[ar, ar, :]
                elif dd == Min // 2:
                    w = (Msym[ar, idx, :] + Msym[idx, ar, :]) * 0.5
                else:
                    w = Msym[ar, idx, :] + Msym[idx, ar, :]
                tiles.append(w)
            sym[nm] = np.stack(tiles, axis=1)                 # [Min, nd, 128]
        rect = np.stack([MX[(ar + c) % Min, ar, :] for c in range(Min)],
                        axis=1)                               # [Min, Min, 128]
        if blk == "b1":
            for nm in ("ss", "vv"):
                sym[nm] = np.concatenate([sym[nm], sym[nm]], axis=0)
            rect = np.concatenate([rect, rect], axis=0)
        bn = blk[1]
        d[f"wss{bn}"] = np.ascontiguousarray(sym["ss"]).astype(np.float16)
        d[f"wvv{bn}"] = np.ascontiguousarray(sym["vv"]).astype(np.float16)
        d[f"wsv{bn}"] = np.ascontiguousarray(rect).astype(np.float16)
        cg = 1.0 / np.sqrt(128)
        for nm, sfx in (("ws", "s"), ("wg", "g"), ("wv", "v")):
            d[f"g{bn}{sfx}"] = (f[f"{blk}_g_{nm}"] * cg).astype(np.float16)
        cog = TANH_GAIN / np.sqrt(128)
        fold[f"O{bn}s"] = f[f"{blk}_o_w0"] * cog
        fold[f"O{bn}v"] = f[f"{blk}_o_w1"] * cog
    cf = 1.0 / np.sqrt(128)
    d["wfs"] = (fold["O2s"] @ (f["f_w0"] * cf)).astype(np.float16)
    d["wfv"] = (fold["O2v"] @ (f["f_w1"] * cf)).astype(np.float16)
    return d


def _make_in_maps(x, w):
    x = np.asarray(x, dtype=np.float32)
    in_maps = []
    for c in range(N_CORES):
        bs = slice(c * BC, (c + 1) * BC)
        xl = x[bs]
        s_loc = np.ascontiguousarray(xl[:, :64].T).astype(np.float16)   # [64, BC]
        v_loc = xl[:, 64:].reshape(BC, 64, 3)
        m = dict(w)
        m["s2d"] = np.concatenate([s_loc, s_loc], axis=0)
        va = np.concatenate([np.ascontiguousarray(v_loc[:, :, i].T)
                             .astype(np.float16) for i in range(3)], axis=1)
        m["v2da"] = np.concatenate([va, va], axis=0)          # [128, 3*BC]
        in_maps.append(m)
    return in_maps


def kernel(**inputs):
    from concourse.bass_utils import run_bass_kernel_spmd

    w = _host_prep(inputs)
    in_maps = _make_in_maps(inputs["x"], w)

    if "nc" not in _CACHE:
        _CACHE["nc"] = _build_program()
    nc = _CACHE["nc"]

    res = run_bass_kernel_spmd(nc, in_maps, core_ids=list(range(N_CORES)))

    out = np.empty((B, 256), dtype=np.float32)
    for c in range(N_CORES):
        o = res.results[c]["out"]                                # [256, BC]
        bs = slice(c * BC, (c + 1) * BC)
        out[bs, :64] = o[:64].T
        v = o[64:].reshape(3, 64, BC)
        out[bs, 64:] = v.transpose(2, 1, 0).reshape(BC, 192)
    return out


# revision 19
# speedup vs baseline: 1.9468x; 1.0375x over previous
"""Trainium2 Bass kernel for nn_EquivariantModel (e3nn-style equivariant net).

Strategy: data-parallel over batch (8 cores x 1024 rows), feature-major
activations.  All o3.Linear layers (l1/l2, block out-linears, final) are
folded host-side into the FullyConnectedTensorProduct weights, so each block
reduces to a bilinear form in its RAW inputs:

    tp_s[b,w] = sum_{pq} s_p s_q MSS[pq,w] + sum_{i,pq} v_ip v_iq MVV[pq,w]
    tp_v[b,w,i] = sum_{pq} s_p v_iq MX[pq,w]

The symmetric forms (s(x)s, v(x)v) need only cyclic diagonals d=0..M/2 (2x
fewer products); products z_d = x * rot_d(x) are built with single
full-width DVE multiplies against partition-rotated copies of x, which are
materialized by grouped DMA reads from a row-doubled DRAM image (one DMA
covers many rotations via an overlapping-stride access pattern; the three
vector components share one image so a group is a single DMA).  Matmuls
stream z through per-diagonal weight tiles, accumulating in PSUM; the ss,
vv and sv streams are interleaved in one d-loop to keep DMA demand flat.
Block 1 (mult 64) stacks two batch halves in the partition dim and uses
split-K matmuls at base partitions 0/64.  A fraction of the products runs
on the GpSimd engine to unload the DVE.
"""

import sys
import numpy as np

if '/opt/trn_rl_repo' not in sys.path:
    sys.path.insert(0, '/opt/trn_rl_repo')

B, M_IN, M_HID = 8192, 64, 128
N_CORES = 8
BC = B // N_CORES            # batch per core
TANH_GAIN = 1.5927116870880127

GRP = 8                      # s-rotations / weight tiles per group DMA
GRPV = 2                     # b2 v-rotation group size (diagonals)
GRPV1 = 1                    # b1 v-rotation group size (pairs)
POOL_EVERY = 5               # every Nth product goes to GpSimd instead of DVE

_CACHE = {}


def _build_program(repeat=1):
    import concourse.mybir as mybir
    import concourse.tile as tile
    from concourse import bacc
    from contextlib import ExitStack
    import bass_rust

    f16 = mybir.dt.float16
    f32 = mybir.dt.float32

    nc = bacc.Bacc("TRN2", target_bir_lowering=False)

    # ---- DRAM I/O ----
    s2d = nc.dram_tensor("s2d", [128, BC], f16, kind="ExternalInput")
    v2da = nc.dram_tensor("v2da", [128, 3 * BC], f16, kind="ExternalInput")
    wss1 = nc.dram_tensor("wss1", [128, 17, 128], f16, kind="ExternalInput")
    wvv1 = nc.dram_tensor("wvv1", [128, 17, 128], f16, kind="ExternalInput")
    wsv1 = nc.dram_tensor("wsv1", [128, 32, 128], f16, kind="ExternalInput")
    wss2 = nc.dram_tensor("wss2", [128, 65, 128], f16, kind="ExternalInput")
    wvv2 = nc.dram_tensor("wvv2", [128, 65, 128], f16, kind="ExternalInput")
    wsv2 = nc.dram_tensor("wsv2", [128, 128, 128], f16, kind="ExternalInput")
    gate_w = {}
    for blk in ("1", "2"):
        for nm in ("s", "g", "v"):
            gate_w[blk + nm] = nc.dram_tensor(f"g{blk}{nm}", [128, 128], f16,
                                              kind="ExternalInput")
    wfs = nc.dram_tensor("wfs", [128, 64], f16, kind="ExternalInput")
    wfv = nc.dram_tensor("wfv", [128, 64], f16, kind="ExternalInput")

    dts = nc.dram_tensor("dts", [256, BC], f16, kind="Internal")
    dgva = nc.dram_tensor("dgva", [256, 3 * BC], f16, kind="Internal")
    out_d = nc.dram_tensor("out", [256, BC], f32, kind="ExternalOutput")

    def src_ap(t, dims, offset):
        s = t[:].copy()
        s.ap = bass_rust.VecI64Pair(dims)
        s.offset = offset
        return s

    with ExitStack() as ctx:
        tc = ctx.enter_context(tile.TileContext(nc))
        consts = ctx.enter_context(tc.tile_pool(name="consts", bufs=1))
        acts = ctx.enter_context(tc.tile_pool(name="acts", bufs=1))
        rotp = ctx.enter_context(tc.tile_pool(name="rot", bufs=2))
        rotv = ctx.enter_context(tc.tile_pool(name="rotv", bufs=4))
        wstr = ctx.enter_context(tc.tile_pool(name="wstr", bufs=2))
        wstv = ctx.enter_context(tc.tile_pool(name="wstv", bufs=3))
        zp = ctx.enter_context(tc.tile_pool(name="z", bufs=8))
        psp = ctx.enter_context(tc.tile_pool(name="ps", bufs=1, space="PSUM"))
        tmp = ctx.enter_context(tc.tile_pool(name="tmp", bufs=1))

        GW = {}
        for k, t in gate_w.items():
            w = consts.tile([128, 128], f16, tag=f"gw{k}", name=f"gw{k}")
            nc.sync.dma_start(w[:], t[:])
            GW[k] = w
        wfs_sb = consts.tile([128, 64], f16, tag="wfs", name="wfs")
        nc.sync.dma_start(wfs_sb[:], wfs[:])
        wfv_sb = consts.tile([128, 64], f16, tag="wfv", name="wfv")
        nc.sync.dma_start(wfv_sb[:], wfv[:])

        nmul = [0]

        def mul(z, a, b):
            nmul[0] += 1
            if POOL_EVERY and nmul[0] % POOL_EVERY == 0:
                nc.gpsimd.tensor_mul(z, a, b)
            else:
                nc.vector.tensor_mul(z, a, b)

        def b1_block():
            # bases: features duplicated across both partition halves
            sb = acts.tile([128, BC], f16, tag="sb1", name="sb1")
            nc.sync.dma_start(sb[0:64, :], s2d[0:64, :])
            nc.sync.dma_start(sb[64:128, :], s2d[0:64, :])
            vb = []
            for i in range(3):
                t = acts.tile([128, BC], f16, tag=f"vb1{i}", name=f"vb1{i}")
                nc.sync.dma_start(t[0:64, :], v2da[0:64, i * BC:(i + 1) * BC])
                nc.sync.dma_start(t[64:128, :], v2da[0:64, i * BC:(i + 1) * BC])
                vb.append(t)

            accs = psp.tile([128, 1024], f32, tag="pa_s", name="pa_s")
            accv = [psp.tile([128, 1024], f32, tag=f"pa_v{i}", name=f"pa_v{i}")
                    for i in range(3)]

            vrot = [None]
            wvv_t = [None]
            NPR = GRP // 2          # diagonal pairs per s-rot group

            # pair (2k, 2k+1): rot tile rows 0:64 = rot_2k, 64:128 = rot_2k+1
            for g0 in range(0, 64, GRP):
                npr = min(NPR, (64 - g0) // 2)
                rot = rotp.tile([128, NPR * BC], f16, tag="rotS1", name="rotS1")
                for h in range(2):
                    nc.sync.dma_start(
                        rot[h * 64:(h + 1) * 64, 0:npr * BC],
                        src_ap(s2d, [[BC, 64], [2 * BC, npr], [1, BC]],
                               (g0 + h) * BC))
                wsv_t = wstr.tile([128, NPR * 128], f16, tag="wsv", name="wsv")
                nc.scalar.dma_start(
                    wsv_t[:, 0:npr * 128],
                    wsv1[:, g0 // 2:g0 // 2 + npr, :].rearrange(
                        "p n m -> p (n m)"))
                nss = max(0, min(npr, 17 - g0 // 2))
                if nss > 0:
                    wss_t = wstr.tile([128, NPR * 128], f16, tag="wss", name="wss")
                    nc.scalar.dma_start(
                        wss_t[:, 0:nss * 128],
                        wss1[:, g0 // 2:g0 // 2 + nss, :].rearrange(
                            "p n m -> p (n m)"))
                for kp in range(npr):
                    gp = g0 // 2 + kp
                    for h in range(2):
                        hs = slice(h * 512, (h + 1) * 512)
                        rj = rot[:, kp * BC + h * 512:kp * BC + (h + 1) * 512]
                        for i in range(3):
                            z = zp.tile([128, 512], f16, tag="z1", name="z1")
                            mul(z, rj, vb[i][:, hs])
                            nc.tensor.matmul(
                                accv[i][:, hs],
                                wsv_t[:, kp * 128:(kp + 1) * 128], z,
                                start=(gp == 0), stop=(gp == 31))
                        if gp > 16:
                            continue
                        z = zp.tile([128, 512], f16, tag="z1", name="z1")
                        mul(z, sb[:, hs], rj)
                        nc.tensor.matmul(accs[:, hs],
                                         wss_t[:, kp * 128:(kp + 1) * 128], z,
                                         start=(gp == 0), stop=False)
                        # interleaved vv stream (pairs x 3 components)
                        if h == 0 and gp % GRPV1 == 0:
                            nv = min(GRPV1, 17 - gp)
                            vr = rotv.tile([128, GRPV1 * 3 * BC], f16,
                                           tag="rotV1", name="rotV1")
                            for hh in range(2):
                                nc.sync.dma_start(
                                    vr[hh * 64:(hh + 1) * 64, 0:nv * 3 * BC],
                                    src_ap(v2da,
                                           [[3 * BC, 64], [2 * 3 * BC, nv],
                                            [BC, 3], [1, BC]],
                                           (2 * gp + hh) * 3 * BC))
                            vrot[0] = vr
                            wt = wstv.tile([128, GRPV * 128], f16, tag="wvv",
                                           name="wvv")
                            nc.scalar.dma_start(
                                wt[:, 0:nv * 128],
                                wvv1[:, gp:gp + nv, :].rearrange(
                                    "p n m -> p (n m)"))
                            wvv_t[0] = wt
                        jv = gp % GRPV1
                        for i in range(3):
                            z = zp.tile([128, 512], f16, tag="z1", name="z1")
                            mul(z, vb[i][:, hs],
                                vrot[0][:, (jv * 3 + i) * BC + h * 512:
                                         (jv * 3 + i) * BC + (h + 1) * 512])
                            nc.tensor.matmul(accs[:, hs],
                                             wvv_t[0][:, jv * 128:(jv + 1) * 128],
                                             z, start=False,
                                             stop=(gp == 16 and i == 2))
            return accs, accv

        def b2_block(tanh_s, gated):
            accs = psp.tile([128, 1024], f32, tag="pa_s", name="pa_s")
            accv = [psp.tile([128, 1024], f32, tag=f"pa_v{i}", name=f"pa_v{i}")
                    for i in range(3)]

            def mm2(acc, w, z, start, stop):
                nc.tensor.matmul(acc[:, 0:512], w, z[:, 0:512],
                                 start=start, stop=stop)
                nc.tensor.matmul(acc[:, 512:1024], w, z[:, 512:1024],
                                 start=start, stop=stop)

            vrot = [None]
            wvv_t = [None]

            for g0 in range(0, 128, GRP):
                ng = min(GRP, 128 - g0)
                rot = rotp.tile([128, GRP * 1024], f16, tag="rotS2", name="rotS2")
                nc.sync.dma_start(
                    rot[:, 0:ng * 1024],
                    src_ap(dts, [[BC, 128], [BC, ng], [1, 1024]], g0 * BC))
                wsv_t = wstr.tile([128, GRP * 128], f16, tag="wsv", name="wsv")
                nc.scalar.dma_start(
                    wsv_t[:, 0:ng * 128],
                    wsv2[:, g0:g0 + ng, :].rearrange("p n m -> p (n m)"))
                nss = max(0, min(ng, 65 - g0))
                if nss > 0:
                    wss_t = wstr.tile([128, GRP * 128], f16, tag="wss", name="wss")
                    nc.scalar.dma_start(
                        wss_t[:, 0:nss * 128],
                        wss2[:, g0:g0 + nss, :].rearrange("p n m -> p (n m)"))
                for j in range(ng):
                    d = g0 + j
                    rj = rot[:, j * 1024:(j + 1) * 1024]
                    for i in range(3):
                        z = zp.tile([128, 1024], f16, tag="z2", name="z2")
                        mul(z, rj, gated[i])
                        mm2(accv[i], wsv_t[:, j * 128:(j + 1) * 128], z,
                            start=(d == 0), stop=(d == 127))
                    if d <= 64:
                        z = zp.tile([128, 1024], f16, tag="z2", name="z2")
                        mul(z, tanh_s, rj)
                        mm2(accs, wss_t[:, j * 128:(j + 1) * 128], z,
                            start=(d == 0), stop=False)
                        if d % GRPV == 0:
                            nv = min(GRPV, 65 - d)
                            vr = rotv.tile([128, GRPV * 3 * 1024], f16,
                                           tag="rotV2", name="rotV2")
                            nc.sync.dma_start(
                                vr[:, 0:nv * 3 * 1024],
                                src_ap(dgva,
                                       [[3 * BC, 128], [3 * BC, nv],
                                        [1, 3 * BC]],
                                       d * 3 * BC))
                            vrot[0] = vr
                            wt = wstv.tile([128, GRPV * 128], f16, tag="wvv",
                                           name="wvv")
                            nc.scalar.dma_start(
                                wt[:, 0:nv * 128],
                                wvv2[:, d:d + nv, :].rearrange(
                                    "p n m -> p (n m)"))
                            wvv_t[0] = wt
                        jv = d % GRPV
                        for i in range(3):
                            z = zp.tile([128, 1024], f16, tag="z2", name="z2")
                            mul(z, gated[i],
                                vrot[0][:, (jv * 3 + i) * 1024:
                                        (jv * 3 + i + 1) * 1024])
                            mm2(accs, wvv_t[0][:, jv * 128:(jv + 1) * 128], z,
                                start=False, stop=(d == 64 and i == 2))
            return accs, accv

        def gate(blk, accs, accv):
            """PSUM accs -> (tanh_s, gated_v[3]) f16 [128, BC]."""
            tp_s = acts.tile([128, BC], f16, tag="tps", name="tps")
            nc.scalar.copy(tp_s[:, :], accs[:, :])
            tp_v = []
            for i in range(3):
                t = acts.tile([128, BC], f16, tag=f"tpv{i}", name=f"tpv{i}")
                nc.scalar.copy(t[:, :], accv[i][:, :])
                tp_v.append(t)
            tanh_s = acts.tile([128, BC], f16, tag=f"ths{blk}", name=f"ths{blk}")
            tg = acts.tile([128, BC], f16, tag="tg", name="tg")
            vl = [acts.tile([128, BC], f16, tag=f"vl{i}", name=f"vl{i}")
                  for i in range(3)]
            # gate matmuls reuse the freed accumulator PSUM slots
            ps = psp.tile([128, 1024], f32, tag="pa_s", name="pa_s")
            psg = psp.tile([128, 1024], f32, tag="pa_v0", name="pa_v0")
            psv = [psp.tile([128, 1024], f32, tag=f"pa_v{i}", name=f"pa_v{i}")
                   for i in (1, 2)]
            psv.append(psp.tile([128, 1024], f32, tag="pa_s", name="pa_s"))
            for h in range(2):
                sl_ = slice(h * 512, (h + 1) * 512)
                nc.tensor.matmul(ps[:, sl_], GW[blk + "s"], tp_s[:, sl_],
                                 start=True, stop=True)
                nc.scalar.activation(tanh_s[:, sl_], ps[:, sl_],
                                     mybir.ActivationFunctionType.Tanh)
                nc.tensor.matmul(psg[:, sl_], GW[blk + "g"], tp_s[:, sl_],
                                 start=True, stop=True)
                nc.scalar.activation(tg[:, sl_], psg[:, sl_],
                                     mybir.ActivationFunctionType.Tanh)
                for i in range(3):
                    nc.tensor.matmul(psv[i][:, sl_], GW[blk + "v"],
                                     tp_v[i][:, sl_], start=True, stop=True)
                    nc.scalar.copy(vl[i][:, sl_], psv[i][:, sl_])
            gated = []
            for i in range(3):
                t = acts.tile([128, BC], f16, tag=f"gv{blk}{i}", name=f"gv{blk}{i}")
                nc.vector.tensor_mul(t, tg, vl[i])
                gated.append(t)
            return tanh_s, gated

        def _network():
            accs, accv = b1_block()
            tanh_s1, gated1 = gate("1", accs, accv)
            # doubled DRAM images for b2 rotations
            nc.sync.dma_start(dts[0:128, :], tanh_s1[:])
            nc.sync.dma_start(dts[128:256, :], tanh_s1[:])
            for i in range(3):
                nc.sync.dma_start(dgva[0:128, i * BC:(i + 1) * BC], gated1[i][:])
                nc.sync.dma_start(dgva[128:256, i * BC:(i + 1) * BC], gated1[i][:])
            accs2, accv2 = b2_block(tanh_s1, gated1)
            tanh_s2, gated2 = gate("2", accs2, accv2)
            # final linears (out-linears folded in)
            fps = psp.tile([128, 1024], f32, tag="pa_v0", name="pa_v0")
            fpv = [psp.tile([128, 1024], f32, tag=t, name=t)
                   for t in ("pa_v1", "pa_v2", "pa_s")]
            for h in range(2):
                sl_ = slice(h * 512, (h + 1) * 512)
                nc.tensor.matmul(fps[0:64, sl_], wfs_sb[:], tanh_s2[:, sl_],
                                 start=True, stop=True)
                ot = tmp.tile([64, 512], f32, tag="outs", name="outs")
                nc.scalar.copy(ot[:, :], fps[0:64, sl_])
                nc.sync.dma_start(out_d[0:64, sl_], ot[:])
                for i in range(3):
                    nc.tensor.matmul(fpv[i][0:64, sl_], wfv_sb[:],
                                     gated2[i][:, sl_], start=True, stop=True)
                    ov = tmp.tile([64, 512], f32, tag=f"outv{i}", name=f"outv{i}")
                    nc.scalar.copy(ov[:, :], fpv[i][0:64, sl_])
                    nc.sync.dma_start(out_d[64 + 64 * i:128 + 64 * i, sl_], ov[:])

        if repeat > 1:
            with tc.For_i(0, repeat, 1):
                _network()
        else:
            _network()

    nc.finalize()
    return nc


def _host_prep(inputs):
    """Fold all linears into TP weights; build diagonal weight streams."""
    f = {k: np.asarray(v, np.float64) for k, v in inputs.items() if k != 'x'}
    d = {}
    fold = {}
    for blk, M in (("b1", 64), ("b2", 128)):
        c1 = 1.0 / np.sqrt(M)
        A, Av = f[f"{blk}_l1_w0"] * c1, f[f"{blk}_l1_w1"] * c1
        Bm, Bv = f[f"{blk}_l2_w0"] * c1, f[f"{blk}_l2_w1"] * c1
        if blk == "b2":
            A, Av = fold["O1s"] @ A, fold["O1v"] @ Av
            Bm, Bv = fold["O1s"] @ Bm, fold["O1v"] @ Bv
        ctp = 1.0 / (M * np.sqrt(2.0))

        def fld(L, R, W, c):
            T = np.tensordot(L, W, axes=(1, 0))
            T = np.tensordot(R, T, axes=(1, 1))
            return c * T.transpose(1, 0, 2)
        MSS = fld(A, Bm, f[f"{blk}_tp_ss"], ctp)
        MVV = fld(Av, Bv, f[f"{blk}_tp_vv"], ctp / np.sqrt(3.0))
        MSV = fld(A, Bv, f[f"{blk}_tp_sv"], ctp)
        MVS = fld(Av, Bm, f[f"{blk}_tp_vs"], ctp)
        MX = MSV + MVS.transpose(1, 0, 2)
        Min = A.shape[0]
        ar = np.arange(Min)
        sym = {}
        for nm, Msym in (("ss", MSS), ("vv", MVV)):
            tiles = []
            for dd in range(Min // 2 + 1):
                idx = (ar + dd) % Min
                if dd == 0:
                    w = Msym[ar, ar, :]
                elif dd == Min // 2:
                    w = (Msym[ar, idx, :] + Msym[idx, ar, :]) * 0.5
                else:
                    w = Msym[ar, idx, :] + Msym[idx, ar, :]
                tiles.append(w)
            sym[nm] = np.stack(tiles, axis=1)                 # [Min, nd, 128]
        rect = np.stack([MX[(ar + c) % Min, ar, :] for c in range(Min)],
                        axis=1)                               # [Min, Min, 128]
        if blk == "b1":
            # pack diagonal pairs (2k, 2k+1) into 128-row tiles; odd counts
            # get a zero-padded bottom half
            def pairs(st):
                nd = st.shape[1]
                tiles = []
                for k in range((nd + 1) // 2):
                    top = st[:, 2 * k, :]
                    bot = (st[:, 2 * k + 1, :] if 2 * k + 1 < nd
                           else np.zeros_like(top))
                    tiles.append(np.concatenate([top, bot], axis=0))
                return np.stack(tiles, axis=1)                # [128, np, 128]
            for nm in ("ss", "vv"):
                sym[nm] = pairs(sym[nm])
            rect = pairs(rect)
        bn = blk[1]
        d[f"wss{bn}"] = np.ascontiguousarray(sym["ss"]).astype(np.float16)
        d[f"wvv{bn}"] = np.ascontiguousarray(sym["vv"]).astype(np.float16)
        d[f"wsv{bn}"] = np.ascontiguousarray(rect).astype(np.float16)
        cg = 1.0 / np.sqrt(128)
        for nm, sfx in (("ws", "s"), ("wg", "g"), ("wv", "v")):
            d[f"g{bn}{sfx}"] = (f[f"{blk}_g_{nm}"] * cg).astype(np.float16)
        cog = TANH_GAIN / np.sqrt(128)
        fold[f"O{bn}s"] = f[f"{blk}_o_w0"] * cog
        fold[f"O{bn}v"] = f[f"{blk}_o_w1"] * cog
    cf = 1.0 / np.sqrt(128)
    d["wfs"] = (fold["O2s"] @ (f["f_w0"] * cf)).astype(np.float16)
    d["wfv"] = (fold["O2v"] @ (f["f_w1"] * cf)).astype(np.float16)
    return d


def _make_in_maps(x, w):
    x = np.asarray(x, dtype=np.float32)
    in_maps = []
    for c in range(N_CORES):
        bs = slice(c * BC, (c + 1) * BC)
        xl = x[bs]
        s_loc = np.ascontiguousarray(xl[:, :64].T).astype(np.float16)   # [64, BC]
        v_loc = xl[:, 64:].reshape(BC, 64, 3)
        m = dict(w)
        m["s2d"] = np.concatenate([s_loc, s_loc], axis=0)
        va = np.concatenate([np.ascontiguousarray(v_loc[:, :, i].T)
                             .astype(np.float16) for i in range(3)], axis=1)
        m["v2da"] = np.concatenate([va, va], axis=0)          # [128, 3*BC]
        in_maps.append(m)
    return in_maps


def kernel(**inputs):
    from concourse.bass_utils import run_bass_kernel_spmd

    w = _host_prep(inputs)
    in_maps = _make_in_maps(inputs["x"], w)

    if "nc" not in _CACHE:
        _CACHE["nc"] = _build_program()
    nc = _CACHE["nc"]

    res = run_bass_kernel_spmd(nc, in_maps, core_ids=list(range(N_CORES)))

    out = np.empty((B, 256), dtype=np.float32)
    for c in range(N_CORES):
        o = res.results[c]["out"]                                # [256, BC]
        bs = slice(c * BC, (c + 1) * BC)
        out[bs, :64] = o[:64].T
        v = o[64:].reshape(3, 64, BC)
        out[bs, 64:] = v.transpose(2, 1, 0).reshape(BC, 192)
    return out
